# revision 1
# baseline (speedup 1.0000x reference)
"""Trainium2 Bass kernel for a 2-layer GraphSAGE (sum aggregation) GNN.

Strategy (8 NeuronCores, SPMD, two launches):
  - Nodes (dst) sharded 12500/core. Edges partitioned by dst owner.
  - Per core, dst nodes are sorted by in-degree (descending) into "ranks";
    ranks tile into 98 blocks of 128 (12544 slots, 44 zero pads).
  - Launch 1: layer-1 aggregation via round-structured [128,1]-indexed
    indirect DMA gathers of 512B x rows, DVE-accumulated into SBUF agg;
    then per tile h = relu(agg @ Wn1 + x @ Ws1 + b1) on PE, and the
    8-wide projections z = h @ Wn2 and o2 = h @ Ws2 + b2.
  - Using segsum(h[src]) @ Wn2 == segsum((h @ Wn2)[src]), only z (8 wide)
    must be exchanged across cores. The host concatenates the per-core z
    shards (collectives are unavailable on this runtime path).
  - Launch 2: layer-2 aggregation gathers 32B z rows with the same round
    structure, adds o2, applies log_softmax; host inverse-permutes rows.

The host side only reshuffles indices / rows (graph partitioning and the
z-shard concat); all feature compute happens on device.
"""

import sys

import numpy as np

sys.path.insert(0, "/opt/trn_rl_repo")

import concourse.bass as bass
import concourse.mybir as mybir
import concourse.tile as tile
from concourse import bacc
from concourse.bass_utils import run_bass_kernel_spmd
from concourse.masks import make_identity

P = 128
N_NODES = 100000
N_CORES = 8
NPC = N_NODES // N_CORES  # 12500
NT = 98  # rank tiles per core
NR = NT * P  # 12544 rank slots per core
NCLS = 8
ZROW_X = N_NODES  # zeros row appended to x gather table
ZROW_Z = NPC  # core-0 pad rank (z value is exactly 0 by construction)
F32 = mybir.dt.float32
I32 = mybir.dt.int32


def _prep_host(x, edge_src, edge_dst):
    """Partition edges by dst core, degree-sort dst ranks, build round-major
    per-tile gather index arrays. Returns per-core arrays + globals."""
    edge_src = np.asarray(edge_src)
    edge_dst = np.asarray(edge_dst)
    core_of = edge_dst // NPC

    orders = []  # per core: rank -> local dst id
    deg_sorted = []  # per core: degree per rank (desc)
    per_core = []
    for k in range(N_CORES):
        m = core_of == k
        s = edge_src[m]
        dl = edge_dst[m] - k * NPC
        deg = np.bincount(dl, minlength=NPC)
        order = np.argsort(-deg, kind="stable")
        rank_of = np.empty(NPC, dtype=np.int64)
        rank_of[order] = np.arange(NPC)
        orders.append(order)
        deg_sorted.append(deg[order])
        per_core.append((s, rank_of[dl]))

    # global z position of each original node (layout of concatenated z)
    zpos = np.empty(N_NODES, dtype=np.int64)
    for k in range(N_CORES):
        zpos[k * NPC + orders[k]] = k * NR + np.arange(NPC)

    # global per-tile round counts (max over cores; >=1)
    R = np.ones(NT, dtype=np.int64)
    for k in range(N_CORES):
        for t in range(NT):
            lead = t * P
            if lead < NPC:
                R[t] = max(R[t], deg_sorted[k][lead])
    off = np.zeros(NT + 1, dtype=np.int64)
    off[1:] = np.cumsum(R)
    TK = int(off[-1])

    I1s, I2s, xTs = [], [], []
    for k in range(N_CORES):
        s, ranks = per_core[k]
        eo = np.argsort(ranks, kind="stable")
        rs = ranks[eo]
        ss = s[eo]
        starts = np.searchsorted(rs, np.arange(NPC))
        occ = np.arange(len(rs)) - starts[rs]
        maxR = int(R.max())
        A1 = np.full((NR, maxR), ZROW_X, dtype=np.int32)
        A2 = np.full((NR, maxR), ZROW_Z, dtype=np.int32)
        A1[rs, occ] = ss
        A2[rs, occ] = zpos[ss]
        I1 = np.empty((P, TK), dtype=np.int32)
        I2 = np.empty((P, TK), dtype=np.int32)
        for t in range(NT):
            blk = slice(t * P, (t + 1) * P)
            I1[:, off[t] : off[t + 1]] = A1[blk, : R[t]]
            I2[:, off[t] : off[t + 1]] = A2[blk, : R[t]]
        I1s.append(I1)
        I2s.append(I2)
        xT = np.zeros((P, NR), dtype=np.float32)
        xT[:, :NPC] = x[k * NPC + orders[k]].T
        xTs.append(np.ascontiguousarray(xT))

    return orders, R, off, TK, I1s, I2s, xTs


def _build_nc1(R, off, TK):
    """Launch 1: layer-1 aggregate + matmuls; outputs z and o2 per core."""
    nc = bacc.Bacc(
        "TRN2", target_bir_lowering=False, debug=False, num_devices=N_CORES
    )
    xg = nc.dram_tensor("xg", [N_NODES + 1, P], F32, kind="ExternalInput").ap()
    xT = nc.dram_tensor("xT", [P, NR], F32, kind="ExternalInput").ap()
    I1 = nc.dram_tensor("I1", [P, TK], I32, kind="ExternalInput").ap()
    W1n = nc.dram_tensor("W1n", [P, P], F32, kind="ExternalInput").ap()
    W1s = nc.dram_tensor("W1s", [P, P], F32, kind="ExternalInput").ap()
    W2n = nc.dram_tensor("W2n", [P, NCLS], F32, kind="ExternalInput").ap()
    W2s = nc.dram_tensor("W2s", [P, NCLS], F32, kind="ExternalInput").ap()
    b1 = nc.dram_tensor("b1", [1, P], F32, kind="ExternalInput").ap()
    b2 = nc.dram_tensor("b2", [1, NCLS], F32, kind="ExternalInput").ap()
    z_k = nc.dram_tensor("z", [P, NT * NCLS], F32, kind="ExternalOutput").ap()
    o2_k = nc.dram_tensor("o2", [P, NT * NCLS], F32, kind="ExternalOutput").ap()

    with tile.TileContext(nc) as tc:
        with (
            tc.tile_pool(name="persist", bufs=1) as pp,
            tc.tile_pool(name="gather", bufs=8) as gp,
            tc.tile_pool(name="work", bufs=3) as wp,
            tc.tile_pool(name="psum", bufs=1, space="PSUM") as psp,
        ):
            w1n = pp.tile([P, P], F32, tag="w1n")
            w1s = pp.tile([P, P], F32, tag="w1s")
            w2n = pp.tile([P, NCLS], F32, tag="w2n")
            w2s = pp.tile([P, NCLS], F32, tag="w2s")
            b1t = pp.tile([1, P], F32, tag="b1")
            b2t = pp.tile([1, NCLS], F32, tag="b2")
            ones = pp.tile([1, P], F32, tag="ones")
            ident = pp.tile([P, P], F32, tag="ident")
            i1t = pp.tile([P, TK], I32, tag="i1")
            xTt = pp.tile([P, NR], F32, tag="xT")
            agg = pp.tile([P, NR], F32, tag="agg")
            zsb = pp.tile([P, NT * NCLS], F32, tag="z")
            o2sb = pp.tile([P, NT * NCLS], F32, tag="o2")

            nc.sync.dma_start(out=w1n[:], in_=W1n[:])
            nc.sync.dma_start(out=w1s[:], in_=W1s[:])
            nc.sync.dma_start(out=w2n[:], in_=W2n[:])
            nc.sync.dma_start(out=w2s[:], in_=W2s[:])
            nc.sync.dma_start(out=b1t[:], in_=b1[:])
            nc.sync.dma_start(out=b2t[:], in_=b2[:])
            nc.sync.dma_start(out=i1t[:], in_=I1[:])
            nc.sync.dma_start(out=xTt[:], in_=xT[:])
            nc.vector.memset(ones[:], 1.0)
            make_identity(nc, ident[:])

            # layer-1 gather+accumulate (tile-major rounds)
            for t in range(NT):
                csl = slice(t * P, (t + 1) * P)
                for r in range(int(R[t])):
                    col = int(off[t]) + r
                    buf = gp.tile([P, P], F32, tag="g1")
                    nc.gpsimd.indirect_dma_start(
                        out=buf[:],
                        out_offset=None,
                        in_=xg[:],
                        in_offset=bass.IndirectOffsetOnAxis(
                            ap=i1t[:, col : col + 1], axis=0
                        ),
                    )
                    if r == 0:
                        nc.vector.tensor_copy(out=agg[:, csl], in_=buf[:])
                    else:
                        nc.vector.tensor_add(
                            out=agg[:, csl], in0=agg[:, csl], in1=buf[:]
                        )

            # per-tile matmuls: h, z, self-path of layer 2
            for t in range(NT):
                csl = slice(t * P, (t + 1) * P)
                zsl = slice(t * NCLS, (t + 1) * NCLS)
                aggT_ps = psp.tile([P, P], F32, tag="aggT_ps")
                nc.tensor.transpose(
                    out=aggT_ps[:], in_=agg[:, csl], identity=ident[:]
                )
                aggT = wp.tile([P, P], F32, tag="aggT")
                nc.vector.tensor_copy(out=aggT[:], in_=aggT_ps[:])
                h_ps = psp.tile([P, P], F32, tag="h_ps")
                nc.tensor.matmul(
                    out=h_ps[:], lhsT=aggT[:], rhs=w1n[:], start=True, stop=False
                )
                nc.tensor.matmul(
                    out=h_ps[:], lhsT=xTt[:, csl], rhs=w1s[:],
                    start=False, stop=False,
                )
                nc.tensor.matmul(
                    out=h_ps[:], lhsT=ones[:1, :], rhs=b1t[:1, :],
                    start=False, stop=True,
                )
                h = wp.tile([P, P], F32, tag="h")
                nc.scalar.activation(
                    out=h[:], in_=h_ps[:], func=mybir.ActivationFunctionType.Relu
                )
                hT_ps = psp.tile([P, P], F32, tag="hT_ps")
                nc.tensor.transpose(out=hT_ps[:], in_=h[:], identity=ident[:])
                hT = wp.tile([P, P], F32, tag="hT")
                nc.vector.tensor_copy(out=hT[:], in_=hT_ps[:])
                z_ps = psp.tile([P, NCLS], F32, tag="z_ps")
                nc.tensor.matmul(
                    out=z_ps[:], lhsT=hT[:], rhs=w2n[:], start=True, stop=True
                )
                nc.vector.tensor_copy(out=zsb[:, zsl], in_=z_ps[:])
                o2_ps = psp.tile([P, NCLS], F32, tag="o2_ps")
                nc.tensor.matmul(
                    out=o2_ps[:], lhsT=hT[:], rhs=w2s[:], start=True, stop=False
                )
                nc.tensor.matmul(
                    out=o2_ps[:], lhsT=ones[:1, :], rhs=b2t[:1, :],
                    start=False, stop=True,
                )
                nc.vector.tensor_copy(out=o2sb[:, zsl], in_=o2_ps[:])

            nc.sync.dma_start(out=z_k, in_=zsb[:])
            nc.sync.dma_start(out=o2_k, in_=o2sb[:])

    nc.compile()
    return nc


def _build_nc2(R, off, TK):
    """Launch 2: layer-2 gather of z rows, add self-path, log_softmax."""
    nc = bacc.Bacc(
        "TRN2", target_bir_lowering=False, debug=False, num_devices=N_CORES
    )
    zf = nc.dram_tensor(
        "zf", [N_CORES * NR, NCLS], F32, kind="ExternalInput"
    ).ap()
    o2_k = nc.dram_tensor("o2", [P, NT * NCLS], F32, kind="ExternalInput").ap()
    I2 = nc.dram_tensor("I2", [P, TK], I32, kind="ExternalInput").ap()
    out = nc.dram_tensor("out", [P, NT * NCLS], F32, kind="ExternalOutput").ap()

    with tile.TileContext(nc) as tc:
        with (
            tc.tile_pool(name="persist", bufs=1) as pp,
            tc.tile_pool(name="gather", bufs=8) as gp,
        ):
            i2t = pp.tile([P, TK], I32, tag="i2")
            o2sb = pp.tile([P, NT * NCLS], F32, tag="o2")
            a2sb = pp.tile([P, NT * NCLS], F32, tag="a2")
            nc.sync.dma_start(out=i2t[:], in_=I2[:])
            nc.sync.dma_start(out=o2sb[:], in_=o2_k[:])

            for t in range(NT):
                zsl = slice(t * NCLS, (t + 1) * NCLS)
                for r in range(int(R[t])):
                    col = int(off[t]) + r
                    buf2 = gp.tile([P, NCLS], F32, tag="g2")
                    nc.gpsimd.indirect_dma_start(
                        out=buf2[:],
                        out_offset=None,
                        in_=zf[:],
                        in_offset=bass.IndirectOffsetOnAxis(
                            ap=i2t[:, col : col + 1], axis=0
                        ),
                    )
                    if r == 0:
                        nc.vector.tensor_copy(out=a2sb[:, zsl], in_=buf2[:])
                    else:
                        nc.vector.tensor_add(
                            out=a2sb[:, zsl], in0=a2sb[:, zsl], in1=buf2[:]
                        )

            nc.vector.tensor_add(out=a2sb[:], in0=a2sb[:], in1=o2sb[:])
            a3 = a2sb[:].rearrange("p (t c) -> p t c", c=NCLS)
            mx = pp.tile([P, NT], F32, tag="mx")
            nc.vector.tensor_reduce(
                out=mx[:], in_=a3, axis=mybir.AxisListType.X,
                op=mybir.AluOpType.max,
            )
            mxb = mx[:].unsqueeze(2).to_broadcast([P, NT, NCLS])
            nc.vector.tensor_tensor(
                out=a3, in0=a3, in1=mxb, op=mybir.AluOpType.subtract
            )
            ex = pp.tile([P, NT * NCLS], F32, tag="ex")
            nc.scalar.activation(
                out=ex[:], in_=a2sb[:], func=mybir.ActivationFunctionType.Exp
            )
            sm = pp.tile([P, NT], F32, tag="sm")
            nc.vector.tensor_reduce(
                out=sm[:],
                in_=ex[:].rearrange("p (t c) -> p t c", c=NCLS),
                axis=mybir.AxisListType.X,
                op=mybir.AluOpType.add,
            )
            lg = pp.tile([P, NT], F32, tag="lg")
            nc.scalar.activation(
                out=lg[:], in_=sm[:], func=mybir.ActivationFunctionType.Ln
            )
            lgb = lg[:].unsqueeze(2).to_broadcast([P, NT, NCLS])
            nc.vector.tensor_tensor(
                out=a3, in0=a3, in1=lgb, op=mybir.AluOpType.subtract
            )
            nc.sync.dma_start(out=out[:], in_=a2sb[:])

    nc.compile()
    return nc


def kernel(
    x, edge_src, edge_dst, W_neigh1, W_self1, b1, W_neigh2, W_self2, b2
):
    x = np.ascontiguousarray(np.asarray(x, dtype=np.float32))
    orders, R, off, TK, I1s, I2s, xTs = _prep_host(x, edge_src, edge_dst)

    xg = np.vstack([x, np.zeros((1, P), np.float32)])
    common = {
        "xg": xg,
        "W1n": np.asarray(W_neigh1, np.float32),
        "W1s": np.asarray(W_self1, np.float32),
        "W2n": np.asarray(W_neigh2, np.float32),
        "W2s": np.asarray(W_self2, np.float32),
        "b1": np.asarray(b1, np.float32).reshape(1, P),
        "b2": np.asarray(b2, np.float32).reshape(1, NCLS),
    }
    in_maps1 = [
        {**common, "xT": xTs[k], "I1": I1s[k]} for k in range(N_CORES)
    ]

    nc1 = _build_nc1(R, off, TK)
    res1 = run_bass_kernel_spmd(nc1, in_maps1, list(range(N_CORES)))

    def _rows(a):  # [P, NT*NCLS] sbuf layout -> [NR, NCLS] rank rows
        return np.ascontiguousarray(
            a.reshape(P, NT, NCLS).transpose(1, 0, 2).reshape(NR, NCLS)
        )

    z_full = np.concatenate(
        [_rows(res1.results[k]["z"]) for k in range(N_CORES)], axis=0
    )
    in_maps2 = [
        {"zf": z_full, "o2": res1.results[k]["o2"], "I2": I2s[k]}
        for k in range(N_CORES)
    ]
    nc2 = _build_nc2(R, off, TK)
    res2 = run_bass_kernel_spmd(nc2, in_maps2, list(range(N_CORES)))

    out_full = np.empty((N_NODES, NCLS), dtype=np.float32)
    for k in range(N_CORES):
        out_full[k * NPC + orders[k]] = _rows(res2.results[k]["out"])[:NPC]
    return out_full


if __name__ == "__main__":
    import jax

    import reference

    cpu = jax.devices("cpu")[0]
    with jax.default_device(cpu):
        inputs = {k: np.asarray(v) for k, v in reference.setup_inputs().items()}
        exp = np.asarray(
            reference.reference(**{k: jax.device_put(v, cpu) for k, v in inputs.items()})
        )
    got = kernel(**inputs)
    err = np.abs(got - exp)
    rel = err / (np.abs(exp) + 1e-6)
    print("max abs err:", err.max(), "max rel err:", rel.max())



# revision 15
# speedup vs baseline: 48.0011x; 48.0011x over previous
"""Trainium2 Bass kernel for a 2-layer GraphSAGE (sum aggregation) GNN.

Strategy (8 NeuronCores, SPMD, two launches):
  - Nodes (dst) sharded 12500/core (identity order); dst tiles of 128.
  - Per-edge neighbor rows are fetched with batched `dma_gather` (InstDMAGatherAnt,
    up to ~2.3k descriptors per instruction across 4 SWDGE queues) from 4
    int16-indexable chunks of the bf16 node-feature table.
  - Edge slots are packed DENSE per (dst-tile, chunk) — no per-rank round
    padding. The dst-segmented reduction is done on the PE: for each piece of
    128 slots, a one-hot selection matrix S (built on DVE from the slot->rank
    vector vs an iota table with is_equal) routes gathered rows into the
    tile's aggT accumulator in PSUM:  aggT[feat,dst] += buf_piece^T-free @ S.
  - Launch 1: aggT -> h = relu(aggT^T@W1n + x@W1s + b1) per tile; writes the
    bf16 h table (layer-2 gather source) and o2 = h@W2s + b2.
  - Host concatenates per-core h tables into 4 chunks (index shuffling only).
  - Launch 2: same dense gather/reduce over h rows -> out = log_softmax(
    aggH^T@W2n + o2).
All engine streams are hand-scheduled (Block style) with explicit semaphores
since the tile framework does not track dma_gather's APs.
"""

import sys

import numpy as np
import ml_dtypes

sys.path.insert(0, "/opt/trn_rl_repo")

import concourse.bass as bass
import concourse.mybir as mybir
from concourse import bacc
from concourse import library_config
from concourse.bass_utils import run_bass_kernel_spmd

P = 128
N_NODES = 100000
N_CORES = 8
NPC = N_NODES // N_CORES  # 12500
SL = 12544  # padded local slots (98 * 128)
NT = SL // P  # 98 dst tiles per core
NCLS = 8
CH = 25088  # chunk rows (4 * CH = 100352 >= N_NODES, and SL * 8 = 100352)
NCH = 4
ZR = CH  # zero-row index within each chunk table
F32 = mybir.dt.float32
BF16 = mybir.dt.bfloat16
I16 = mybir.dt.int16
BF = ml_dtypes.bfloat16


def _plan(k, t, r, c_arr, l_arr, sortkey):
    """Dense chunk-grouped slot plan shared by both launches.

    Returns global block sizes N[t][c] (max over cores, padded to 128),
    per-tile piece counts, and per-core wrapped idx + slot->rank arrays.
    """
    key = (k * NT + t) * NCH + c_arr
    cnt = np.bincount(key, minlength=N_CORES * NT * NCH).reshape(
        N_CORES, NT, NCH
    )
    N = cnt.max(axis=0)
    N = ((N + 127) // 128) * 128  # [NT, NCH]
    boff = np.zeros((NT, NCH + 1), np.int64)
    boff[:, 1:] = np.cumsum(N, axis=1)
    slots = boff[:, -1]
    TP = slots // 128
    pbase = np.zeros(NT + 1, np.int64)
    pbase[1:] = np.cumsum(TP)
    slotbase = 128 * pbase[:NT]
    TOT = int(128 * pbase[-1])

    idxs, rankvs = [], []
    for kk in range(N_CORES):
        m = np.flatnonzero(k == kk)
        tt = t[m]
        cc = c_arr[m]
        ll = l_arr[m]
        rr = r[m]
        order = np.lexsort((sortkey[m], cc, tt))
        tt, cc, ll, rr = tt[order], cc[order], ll[order], rr[order]
        g = tt * NCH + cc
        firsts = np.r_[0, np.flatnonzero(np.diff(g)) + 1]
        start_of = np.zeros(len(g), np.int64)
        start_of[firsts] = firsts
        start_of = np.maximum.accumulate(start_of)
        cumc = np.arange(len(g)) - start_of
        slot = slotbase[tt] + boff[tt, cc] + cumc
        idx_arr = np.full(TOT, ZR, np.int16)
        idx_arr[slot] = ll.astype(np.int16)
        rank_arr = np.zeros(TOT, np.float32)
        rank_arr[slot] = rr
        iw = np.tile(np.ascontiguousarray(idx_arr.reshape(-1, 16).T), (8, 1))
        rankv = np.ascontiguousarray(rank_arr.reshape(-1, 128).T.astype(BF))
        idxs.append(np.ascontiguousarray(iw))
        rankvs.append(rankv)
    return dict(
        N=N, boff=boff, slots=slots, TP=TP, pbase=pbase, TOT=TOT,
        idxs=idxs, rankvs=rankvs,
    )


def _prep_host(x, edge_src, edge_dst):
    es = np.asarray(edge_src).astype(np.int64)
    ed = np.asarray(edge_dst).astype(np.int64)
    k = ed // NPC
    d = ed - k * NPC
    t = d >> 7
    r = (d & 127).astype(np.float32)

    c1 = es // CH
    l1 = es - c1 * CH
    sk = es // NPC
    hrow = sk * SL + (es - sk * NPC)
    c2 = hrow // CH
    l2 = hrow - c2 * CH

    plan1 = _plan(k, t, r, c1, l1, es)
    plan2 = _plan(k, t, r, c2, l2, hrow)

    x = np.ascontiguousarray(np.asarray(x, np.float32))
    xpad = np.zeros((NCH * CH, P), np.float32)
    xpad[:N_NODES] = x
    xc = np.zeros((NCH, CH + 1, P), np.float32)
    for c in range(NCH):
        xc[c, :CH] = xpad[c * CH : (c + 1) * CH]

    xTs = []
    for kk in range(N_CORES):
        xT = np.zeros((P, SL), np.float32)
        xT[:, :NPC] = x[kk * NPC : (kk + 1) * NPC].T
        xTs.append(np.ascontiguousarray(xT))

    return plan1, plan2, xc, xTs


SWIN = 3  # S-build lookahead window (tiles)


def _ring_params(plan, elem_bytes):
    MAXSLAB = int(plan["slots"].max())
    RING = max(4, min(12, (44 * 1024) // (MAXSLAB * elem_bytes)))
    TP = plan["TP"]
    SR = 0
    for t in range(NT):
        SR = max(SR, int(TP[max(0, t - (SWIN - 1)) : t + 1].sum()))
    SRING = SR + 2
    return MAXSLAB, RING, SRING


def _emit_gathers(gpsimd, plan, ring_s, iw_s, chunk_tabs, gsems, agd, RING,
                  MAXSLAB):
    N, boff, pbase = plan["N"], plan["boff"], plan["pbase"]
    for t in range(NT):
        if t >= RING:
            gpsimd.wait_ge(agd, t - RING + 1)
        base = (t % RING) * MAXSLAB
        for c in range(NCH):
            n = int(N[t, c])
            ic = int((128 * pbase[t] + boff[t, c]) // 16)
            bo = int(boff[t, c])
            gpsimd.dma_gather(
                ring_s[:, base + bo : base + bo + n].rearrange(
                    "p (j e) -> p j e", e=P
                ),
                chunk_tabs[c][:],
                iw_s[:, ic : ic + n // 16],
                n,
                n,
                P,
                transpose=False,
                single_packet=False,
                queue_num=c,
            ).then_inc(gsems[c][t % RING], 16)


def _emit_sbuild(vector, plan, S_s, rkv_s, iota_s, t, SRING, ssem):
    TP, pbase = plan["TP"], plan["pbase"]
    for j in range(int(TP[t])):
        pi = int(pbase[t]) + j
        sl = (pi % SRING) * P
        vector.tensor_tensor(
            out=S_s[:, sl : sl + P],
            in0=rkv_s[:, pi : pi + 1].to_broadcast([P, P]),
            in1=iota_s[:],
            op=mybir.AluOpType.is_equal,
        ).then_inc(ssem, 1)


def _emit_pieces(tensor, plan, ring_s, S_s, agg_ps, t, RING, MAXSLAB, SRING,
                 ssem, agd):
    TP, pbase = plan["TP"], plan["pbase"]
    base = (t % RING) * MAXSLAB
    ntp = int(TP[t])
    for j in range(ntp):
        pi = int(pbase[t]) + j
        tensor.wait_ge(ssem, pi + 1)
        mm = tensor.matmul(
            out=agg_ps[t % 2][:],
            lhsT=ring_s[:, base + j * P : base + (j + 1) * P],
            rhs=S_s[:, (pi % SRING) * P : (pi % SRING) * P + P],
            start=(j == 0),
            stop=(j == ntp - 1),
        )
    mm.then_inc(agd, 1)


def _build_nc1(plan1):
    MAXSLAB, RING, SRING = _ring_params(plan1, 4)
    TOT, NP1 = plan1["TOT"], int(plan1["pbase"][-1])

    nc = bacc.Bacc(
        "TRN2", target_bir_lowering=False, debug=False,
        num_devices=N_CORES, num_swdge_queues=4,
    )
    xc_d = [
        nc.dram_tensor(f"xc{c}", [CH + 1, P], F32, kind="ExternalInput")
        for c in range(NCH)
    ]
    xT_d = nc.dram_tensor("xT", [P, SL], F32, kind="ExternalInput")
    iw_d = nc.dram_tensor("i1w", [P, TOT // 16], I16, kind="ExternalInput")
    rkv_d = nc.dram_tensor("rkv", [P, NP1], BF16, kind="ExternalInput")
    iota_d = nc.dram_tensor("iota", [P, P], BF16, kind="ExternalInput")
    ident_d = nc.dram_tensor("ident", [P, P], F32, kind="ExternalInput")
    w1n_d = nc.dram_tensor("W1n", [P, P], F32, kind="ExternalInput")
    w1s_d = nc.dram_tensor("W1s", [P, P], F32, kind="ExternalInput")
    w2s_d = nc.dram_tensor("W2s", [P, NCLS], F32, kind="ExternalInput")
    b1_d = nc.dram_tensor("b1", [1, P], F32, kind="ExternalInput")
    b2_d = nc.dram_tensor("b2", [1, NCLS], F32, kind="ExternalInput")
    ones_d = nc.dram_tensor("ones", [1, P], F32, kind="ExternalInput")
    h_d = nc.dram_tensor("h", [SL, P], F32, kind="ExternalOutput")
    o2_d = nc.dram_tensor("o2", [P, NT * NCLS], F32, kind="ExternalOutput")

    from contextlib import ExitStack
    with ExitStack() as ctx:
        block = ctx.enter_context(nc.Block())
        sb = lambda *a: ctx.enter_context(nc.sbuf_tensor(*a))
        ps = lambda *a: ctx.enter_context(nc.psum_tensor(*a))
        sem = lambda n: ctx.enter_context(nc.semaphore(n))
        iw_s = sb("iw_s", [P, TOT // 16], I16)
        xT_s = sb("xT_s", [P, SL], F32)
        rkv_s = sb("rkv_s", [P, NP1], BF16)
        iota_s = sb("iota_s", [P, P], BF16)
        ident_s = sb("ident_s", [P, P], F32)
        w1n_s = sb("w1n_s", [P, P], F32)
        w1s_s = sb("w1s_s", [P, P], F32)
        w2s_s = sb("w2s_s", [P, NCLS], F32)
        b1_s = sb("b1_s", [1, P], F32)
        b2_s = sb("b2_s", [1, NCLS], F32)
        ones_s = sb("ones_s", [1, P], F32)
        ring_s = sb("ring_s", [P, RING * MAXSLAB], F32)
        S_s = sb("S_s", [P, SRING * P], F32)
        aggT_s = sb("aggT_s", [P, 2 * P], F32)
        h_s = sb("h_s", [P, 2 * P], F32)
        hT_s = sb("hT_s", [P, 2 * P], F32)
        o2_sb = sb("o2_sb", [P, NT * NCLS], F32)
        agg_ps = [ps("agg_ps0", [P, P], F32), ps("agg_ps1", [P, P], F32)]
        h_ps = [ps("h_ps0", [P, P], F32), ps("h_ps1", [P, P], F32)]
        hT_ps = [ps("hT_ps0", [P, P], F32), ps("hT_ps1", [P, P], F32)]
        o2_ps = [ps("o2_ps0", [P, NCLS], F32), ps("o2_ps1", [P, NCLS], F32)]
        ld = sem("ld"); ss = sem("ss"); agd = sem("agd")
        gsems = [
            [sem(f"gs{c}_{w}") for w in range(RING)] for c in range(NCH)
        ]
        agc = sem("agc"); hd = sem("hd"); hr = sem("hr")
        hw2 = [sem("hw0"), sem("hw1")]; ow = sem("ow")
        htd = sem("htd"); htc = sem("htc"); o2d = sem("o2d"); o2c = sem("o2c")
        loads = [
            (iw_s, iw_d), (xT_s, xT_d), (rkv_s, rkv_d), (iota_s, iota_d),
            (ident_s, ident_d), (w1n_s, w1n_d), (w1s_s, w1s_d),
            (w2s_s, w2s_d), (b1_s, b1_d), (b2_s, b2_d), (ones_s, ones_d),
        ]
        NL = len(loads)

        @block.sync
        def _(sync: bass.BassEngine):
            for dst, src in loads:
                sync.dma_start(dst[:], src[:]).then_inc(ld, 16)
            for t in range(NT):
                sync.wait_ge(hr, t + 1)
                sync.dma_start(
                    h_d[t * P : (t + 1) * P, :],
                    h_s[:, (t % 2) * P : (t % 2 + 1) * P],
                ).then_inc(hw2[t % 2], 16)
            sync.wait_ge(o2c, NT)
            sync.dma_start(o2_d[:], o2_sb[:]).then_inc(ow, 16)
            sync.wait_ge(hw2[0], 16 * ((NT + 1) // 2))
            sync.wait_ge(hw2[1], 16 * (NT // 2))
            sync.wait_ge(ow, 16)

        @block.gpsimd
        def _(gpsimd: bass.BassGpSimd):
            gpsimd.load_library(library_config.mlp)
            gpsimd.wait_ge(ld, 16 * NL)
            _emit_gathers(
                gpsimd, plan1, ring_s, iw_s, xc_d, gsems, agd, RING, MAXSLAB
            )

        @block.vector
        def _(vector: bass.BassVectorEngine):
            vector.wait_ge(ld, 16 * NL)
            for u in range(NT + 4):
                if u < NT:
                    if u >= SWIN:
                        vector.wait_ge(agd, u - (SWIN - 1))
                    _emit_sbuild(
                        vector, plan1, S_s, rkv_s, iota_s, u, SRING, ss
                    )
                if 1 <= u <= NT:
                    t = u - 1
                    vector.wait_ge(agd, t + 1)
                    vector.tensor_copy(
                        out=aggT_s[:, (t % 2) * P : (t % 2 + 1) * P],
                        in_=agg_ps[t % 2][:],
                    ).then_inc(agc, 1)
                if 3 <= u <= NT + 2:
                    t = u - 3
                    vector.wait_ge(htd, t + 1)
                    vector.tensor_copy(
                        out=hT_s[:, (t % 2) * P : (t % 2 + 1) * P],
                        in_=hT_ps[t % 2][:],
                    ).then_inc(htc, 1)
                if 4 <= u <= NT + 3:
                    t = u - 4
                    vector.wait_ge(o2d, t + 1)
                    vector.tensor_copy(
                        out=o2_sb[:, t * NCLS : (t + 1) * NCLS],
                        in_=o2_ps[t % 2][:],
                    ).then_inc(o2c, 1)

        @block.tensor
        def _(tensor: bass.BassEngine):
            tensor.wait_ge(ld, 16 * NL)
            for u in range(NT + 3):
                if u < NT:
                    t = u
                    if t >= 2:
                        tensor.wait_ge(agc, t - 1)
                    for c in range(NCH):
                        tensor.wait_ge(
                            gsems[c][t % RING], 16 * (t // RING + 1)
                        )
                    _emit_pieces(
                        tensor, plan1, ring_s, S_s, agg_ps, t, RING,
                        MAXSLAB, SRING, ss, agd,
                    )
                if 1 <= u <= NT:
                    t = u - 1
                    tensor.wait_ge(agc, t + 1)
                    if t >= 2:
                        tensor.wait_ge(hr, t - 1)
                    hps = h_ps[t % 2][:]
                    tensor.matmul(
                        out=hps,
                        lhsT=aggT_s[:, (t % 2) * P : (t % 2 + 1) * P],
                        rhs=w1n_s[:], start=True, stop=False,
                    )
                    tensor.matmul(
                        out=hps, lhsT=xT_s[:, t * P : (t + 1) * P],
                        rhs=w1s_s[:], start=False, stop=False,
                    )
                    tensor.matmul(
                        out=hps, lhsT=ones_s[:1, :], rhs=b1_s[:1, :],
                        start=False, stop=True,
                    ).then_inc(hd, 1)
                if 2 <= u <= NT + 1:
                    t = u - 2
                    tensor.wait_ge(hr, t + 1)
                    if t >= 2:
                        tensor.wait_ge(htc, t - 1)
                    # transpose h via matmul with identity rhs:
                    # out[m,n] = sum_p h[p,m] * I[p,n] = h[n,m]
                    tensor.matmul(
                        out=hT_ps[t % 2][:],
                        lhsT=h_s[:, (t % 2) * P : (t % 2 + 1) * P],
                        rhs=ident_s[:], start=True, stop=True,
                    ).then_inc(htd, 1)
                if 3 <= u <= NT + 2:
                    t = u - 3
                    tensor.wait_ge(htc, t + 1)
                    if t >= 2:
                        tensor.wait_ge(o2c, t - 1)
                    ops = o2_ps[t % 2][:]
                    tensor.matmul(
                        out=ops,
                        lhsT=hT_s[:, (t % 2) * P : (t % 2 + 1) * P],
                        rhs=w2s_s[:], start=True, stop=False,
                    )
                    tensor.matmul(
                        out=ops, lhsT=ones_s[:1, :], rhs=b2_s[:1, :],
                        start=False, stop=True,
                    ).then_inc(o2d, 1)

        @block.scalar
        def _(scalar: bass.BassEngine):
            for t in range(NT):
                scalar.wait_ge(hd, t + 1)
                if t >= 2:
                    scalar.wait_ge(hw2[t % 2], 16 * ((t - 2) // 2 + 1))
                    scalar.wait_ge(htd, t - 1)
                scalar.activation(
                    out=h_s[:, (t % 2) * P : (t % 2 + 1) * P],
                    in_=h_ps[t % 2][:],
                    func=mybir.ActivationFunctionType.Relu,
                ).then_inc(hr, 1)

    nc.compile()
    return nc


def _build_nc2(plan2):
    MAXSLAB, RING, SRING = _ring_params(plan2, 4)
    TOT, NP2 = plan2["TOT"], int(plan2["pbase"][-1])

    nc = bacc.Bacc(
        "TRN2", target_bir_lowering=False, debug=False,
        num_devices=N_CORES, num_swdge_queues=4,
    )
    hc_d = [
        nc.dram_tensor(f"hc{c}", [CH + 1, P], F32, kind="ExternalInput")
        for c in range(NCH)
    ]
    iw_d = nc.dram_tensor("i2w", [P, TOT // 16], I16, kind="ExternalInput")
    rkv_d = nc.dram_tensor("rkv2", [P, NP2], F32, kind="ExternalInput")
    iota_d = nc.dram_tensor("iota", [P, P], F32, kind="ExternalInput")
    w2n_d = nc.dram_tensor("W2n", [P, NCLS], F32, kind="ExternalInput")
    o2_d = nc.dram_tensor("o2", [P, NT * NCLS], F32, kind="ExternalInput")
    out_d = nc.dram_tensor("out", [P, NT * NCLS], F32, kind="ExternalOutput")

    from contextlib import ExitStack
    with ExitStack() as ctx:
        block = ctx.enter_context(nc.Block())
        sb = lambda *a: ctx.enter_context(nc.sbuf_tensor(*a))
        ps = lambda *a: ctx.enter_context(nc.psum_tensor(*a))
        sem = lambda n: ctx.enter_context(nc.semaphore(n))
        iw_s = sb("iw_s", [P, TOT // 16], I16)
        rkv_s = sb("rkv_s", [P, NP2], F32)
        iota_s = sb("iota_s", [P, P], F32)
        w2n_s = sb("w2n_s", [P, NCLS], F32)
        o2_sb = sb("o2_sb", [P, NT * NCLS], F32)
        ring_s = sb("ring_s", [P, RING * MAXSLAB], F32)
        S_s = sb("S_s", [P, SRING * P], F32)
        aggH_s = sb("aggH_s", [P, 2 * P], F32)
        a_sb = sb("a_sb", [P, NT * NCLS], F32)
        ex_sb = sb("ex_sb", [P, NT * NCLS], F32)
        mx_sb = sb("mx_sb", [P, NT], F32)
        sm_sb = sb("sm_sb", [P, NT], F32)
        lg_sb = sb("lg_sb", [P, NT], F32)
        agg_ps = [ps("agg_ps0", [P, P], F32), ps("agg_ps1", [P, P], F32)]
        y_ps = [ps("y_ps0", [P, NCLS], F32), ps("y_ps1", [P, NCLS], F32)]
        ld = sem("ld"); ss = sem("ss"); agd = sem("agd")
        gsems = [
            [sem(f"gs{c}_{w}") for w in range(RING)] for c in range(NCH)
        ]
        agc = sem("agc"); yd = sem("yd"); dva = sem("dva"); dvm = sem("dvm")
        ae = sem("ae"); dvs = sem("dvs"); al = sem("al"); dvf = sem("dvf")
        mxd = sem("mxd")
        ow = sem("ow")
        loads = [
            (iw_s, iw_d), (rkv_s, rkv_d), (iota_s, iota_d),
            (w2n_s, w2n_d), (o2_sb, o2_d),
        ]
        NL = len(loads)

        @block.sync
        def _(sync: bass.BassEngine):
            for dst, src in loads:
                sync.dma_start(dst[:], src[:]).then_inc(ld, 16)
            sync.wait_ge(dvf, 1)
            sync.dma_start(out_d[:], a_sb[:]).then_inc(ow, 16)
            sync.wait_ge(ow, 16)

        @block.gpsimd
        def _(gpsimd: bass.BassGpSimd):
            gpsimd.load_library(library_config.mlp)
            gpsimd.wait_ge(ld, 16 * NL)
            _emit_gathers(
                gpsimd, plan2, ring_s, iw_s, hc_d, gsems, agd, RING, MAXSLAB
            )

        @block.vector
        def _(vector: bass.BassVectorEngine):
            vector.wait_ge(ld, 16 * NL)
            for u in range(NT + 2):
                if u < NT:
                    if u >= SWIN:
                        vector.wait_ge(agd, u - (SWIN - 1))
                    _emit_sbuild(
                        vector, plan2, S_s, rkv_s, iota_s, u, SRING, ss
                    )
                if 1 <= u <= NT:
                    t = u - 1
                    vector.wait_ge(agd, t + 1)
                    vector.tensor_copy(
                        out=aggH_s[:, (t % 2) * P : (t % 2 + 1) * P],
                        in_=agg_ps[t % 2][:],
                    ).then_inc(agc, 1)
                if 2 <= u <= NT + 1:
                    t = u - 2
                    vector.wait_ge(yd, t + 1)
                    vector.tensor_tensor(
                        out=a_sb[:, t * NCLS : (t + 1) * NCLS],
                        in0=y_ps[t % 2][:],
                        in1=o2_sb[:, t * NCLS : (t + 1) * NCLS],
                        op=mybir.AluOpType.add,
                    ).then_inc(dva, 1)
            # log-softmax tail over the whole [P, NT, NCLS] array
            vector.wait_ge(dva, NT)
            a3 = a_sb[:].rearrange("p (t c) -> p t c", c=NCLS)
            vector.tensor_reduce(
                out=mx_sb[:], in_=a3, axis=mybir.AxisListType.X,
                op=mybir.AluOpType.max,
            ).then_inc(mxd, 1)
            vector.wait_ge(mxd, 1)
            mxb = mx_sb[:].unsqueeze(2).to_broadcast([P, NT, NCLS])
            vector.tensor_tensor(
                out=a3, in0=a3, in1=mxb, op=mybir.AluOpType.subtract
            ).then_inc(dvm, 1)
            vector.wait_ge(ae, 1)
            vector.tensor_reduce(
                out=sm_sb[:],
                in_=ex_sb[:].rearrange("p (t c) -> p t c", c=NCLS),
                axis=mybir.AxisListType.X,
                op=mybir.AluOpType.add,
            ).then_inc(dvs, 1)
            vector.wait_ge(al, 1)
            lgb = lg_sb[:].unsqueeze(2).to_broadcast([P, NT, NCLS])
            vector.tensor_tensor(
                out=a3, in0=a3, in1=lgb, op=mybir.AluOpType.subtract
            ).then_inc(dvf, 1)

        @block.tensor
        def _(tensor: bass.BassEngine):
            tensor.wait_ge(ld, 16 * NL)
            for u in range(NT + 1):
                if u < NT:
                    t = u
                    if t >= 2:
                        tensor.wait_ge(agc, t - 1)
                    for c in range(NCH):
                        tensor.wait_ge(
                            gsems[c][t % RING], 16 * (t // RING + 1)
                        )
                    _emit_pieces(
                        tensor, plan2, ring_s, S_s, agg_ps, t, RING,
                        MAXSLAB, SRING, ss, agd,
                    )
                if 1 <= u <= NT:
                    t = u - 1
                    tensor.wait_ge(agc, t + 1)
                    if t >= 2:
                        tensor.wait_ge(dva, t - 1)
                    tensor.matmul(
                        out=y_ps[t % 2][:],
                        lhsT=aggH_s[:, (t % 2) * P : (t % 2 + 1) * P],
                        rhs=w2n_s[:], start=True, stop=True,
                    ).then_inc(yd, 1)

        @block.scalar
        def _(scalar: bass.BassEngine):
            scalar.wait_ge(dvm, 1)
            scalar.activation(
                out=ex_sb[:], in_=a_sb[:],
                func=mybir.ActivationFunctionType.Exp,
            ).then_inc(ae, 1)
            scalar.wait_ge(dvs, 1)
            scalar.activation(
                out=lg_sb[:], in_=sm_sb[:],
                func=mybir.ActivationFunctionType.Ln,
            ).then_inc(al, 1)

    nc.compile()
    return nc


def _common_inputs1(plan1, xc, xTs, W_neigh1, W_self1, b1, W_neigh2, W_self2,
                    b2):
    iota = np.ascontiguousarray(
        np.broadcast_to(np.arange(P, dtype=np.float32), (P, P)).astype(BF)
    )
    ident = np.eye(P, dtype=np.float32)
    common = {
        **{f"xc{c}": np.ascontiguousarray(xc[c]) for c in range(NCH)},
        "iota": iota,
        "ident": ident,
        "W1n": np.asarray(W_neigh1, np.float32),
        "W1s": np.asarray(W_self1, np.float32),
        "W2s": np.asarray(W_self2, np.float32),
        "b1": np.asarray(b1, np.float32).reshape(1, P),
        "b2": np.asarray(b2, np.float32).reshape(1, NCLS),
        "ones": np.ones((1, P), np.float32),
    }
    return [
        {
            **common,
            "xT": xTs[k],
            "i1w": plan1["idxs"][k],
            "rkv": plan1["rankvs"][k],
        }
        for k in range(N_CORES)
    ]


def _inputs2(plan2, hc, o2s, W_neigh2):
    iota = np.ascontiguousarray(
        np.broadcast_to(np.arange(P, dtype=np.float32), (P, P)).copy()
    )
    common = {
        **{f"hc{c}": np.ascontiguousarray(hc[c]) for c in range(NCH)},
        "iota": iota,
        "W2n": np.asarray(W_neigh2, np.float32),
    }
    return [
        {
            **common,
            "i2w": plan2["idxs"][k],
            "rkv2": plan2["rankvs"][k].astype(np.float32),
            "o2": o2s[k],
        }
        for k in range(N_CORES)
    ]


def _h_chunks(h_list):
    htab = np.concatenate(h_list, axis=0)  # [8*SL, P] == [4*CH, P]
    hc = np.zeros((NCH, CH + 1, P), np.float32)
    for c in range(NCH):
        hc[c, :CH] = htab[c * CH : (c + 1) * CH]
    return hc


def kernel(
    x, edge_src, edge_dst, W_neigh1, W_self1, b1, W_neigh2, W_self2, b2
):
    plan1, plan2, xc, xTs = _prep_host(x, edge_src, edge_dst)

    in_maps1 = _common_inputs1(
        plan1, xc, xTs, W_neigh1, W_self1, b1, W_neigh2, W_self2, b2
    )
    nc1 = _build_nc1(plan1)
    res1 = run_bass_kernel_spmd(nc1, in_maps1, list(range(N_CORES)))

    hc = _h_chunks([res1.results[k]["h"] for k in range(N_CORES)])
    in_maps2 = _inputs2(
        plan2, hc, [res1.results[k]["o2"] for k in range(N_CORES)], W_neigh2
    )
    nc2 = _build_nc2(plan2)
    res2 = run_bass_kernel_spmd(nc2, in_maps2, list(range(N_CORES)))

    out_full = np.empty((N_NODES, NCLS), dtype=np.float32)
    for k in range(N_CORES):
        o = res2.results[k]["out"]
        rows = o.reshape(P, NT, NCLS).transpose(1, 0, 2).reshape(SL, NCLS)
        out_full[k * NPC : (k + 1) * NPC] = rows[:NPC]
    return out_full


if __name__ == "__main__":
    import jax

    import reference

    cpu = jax.devices("cpu")[0]
    with jax.default_device(cpu):
        inputs = {
            k: np.asarray(v) for k, v in reference.setup_inputs().items()
        }
        exp = np.asarray(
            reference.reference(
                **{k: jax.device_put(v, cpu) for k, v in inputs.items()}
            )
        )
    got = kernel(**inputs)
    err = np.abs(got - exp)
    denom = np.maximum(np.abs(exp), 1e-3)
    print("max abs err:", err.max(), "max rel err:", (err / denom).max())


# revision 16
# speedup vs baseline: 99.9708x; 2.0827x over previous
"""Trainium2 Bass kernel for a 2-layer GraphSAGE (sum aggregation) GNN.

Strategy (8 NeuronCores, SPMD, two launches):
  - Nodes (dst) sharded 12500/core (identity order); dst tiles of 128.
  - Per-edge neighbor rows are fetched with batched `dma_gather` (InstDMAGatherAnt,
    up to ~2.3k descriptors per instruction across 4 SWDGE queues) from 4
    int16-indexable chunks of the bf16 node-feature table.
  - Edge slots are packed DENSE per (dst-tile, chunk) — no per-rank round
    padding. The dst-segmented reduction is done on the PE: for each piece of
    128 slots, a one-hot selection matrix S (built on DVE from the slot->rank
    vector vs an iota table with is_equal) routes gathered rows into the
    tile's aggT accumulator in PSUM:  aggT[feat,dst] += buf_piece^T-free @ S.
  - Launch 1: aggT -> h = relu(aggT^T@W1n + x@W1s + b1) per tile; writes the
    bf16 h table (layer-2 gather source) and o2 = h@W2s + b2.
  - Host concatenates per-core h tables into 4 chunks (index shuffling only).
  - Launch 2: same dense gather/reduce over h rows -> out = log_softmax(
    aggH^T@W2n + o2).
All engine streams are hand-scheduled (Block style) with explicit semaphores
since the tile framework does not track dma_gather's APs.
"""

import sys

import numpy as np
import ml_dtypes

sys.path.insert(0, "/opt/trn_rl_repo")

import concourse.bass as bass
import concourse.mybir as mybir
from concourse import bacc
from concourse import library_config
from concourse.bass_utils import run_bass_kernel_spmd

P = 128
N_NODES = 100000
N_CORES = 8
NPC = N_NODES // N_CORES  # 12500
SL = 12544  # padded local slots (98 * 128)
NT = SL // P  # 98 dst tiles per core
NCLS = 8
CH = 25088  # chunk rows (4 * CH = 100352 >= N_NODES, and SL * 8 = 100352)
NCH = 4
ZR = CH  # zero-row index within each chunk table
F32 = mybir.dt.float32
BF16 = mybir.dt.bfloat16
I16 = mybir.dt.int16
BF = ml_dtypes.bfloat16


def _plan(k, t, r, c_arr, l_arr, sortkey):
    """Dense chunk-grouped slot plan shared by both launches.

    Returns global block sizes N[t][c] (max over cores, padded to 128),
    per-tile piece counts, and per-core wrapped idx + slot->rank arrays.
    """
    key = (k * NT + t) * NCH + c_arr
    cnt = np.bincount(key, minlength=N_CORES * NT * NCH).reshape(
        N_CORES, NT, NCH
    )
    N = cnt.max(axis=0)
    N = ((N + 127) // 128) * 128  # [NT, NCH]
    boff = np.zeros((NT, NCH + 1), np.int64)
    boff[:, 1:] = np.cumsum(N, axis=1)
    slots = boff[:, -1]
    TP = slots // 128
    pbase = np.zeros(NT + 1, np.int64)
    pbase[1:] = np.cumsum(TP)
    slotbase = 128 * pbase[:NT]
    TOT = int(128 * pbase[-1])

    idxs, rankvs, cnts = [], [], []
    for kk in range(N_CORES):
        m = np.flatnonzero(k == kk)
        tt = t[m]
        cc = c_arr[m]
        ll = l_arr[m]
        rr = r[m]
        order = np.lexsort((sortkey[m], cc, tt))
        tt, cc, ll, rr = tt[order], cc[order], ll[order], rr[order]
        g = tt * NCH + cc
        firsts = np.r_[0, np.flatnonzero(np.diff(g)) + 1]
        start_of = np.zeros(len(g), np.int64)
        start_of[firsts] = firsts
        start_of = np.maximum.accumulate(start_of)
        cumc = np.arange(len(g)) - start_of
        slot = slotbase[tt] + boff[tt, cc] + cumc
        idx_arr = np.full(TOT, -1, np.int16)
        idx_arr[slot] = ll.astype(np.int16)
        rank_arr = np.full(TOT, 128.0, np.float32)
        rank_arr[slot] = rr
        # per-(t,c) true counts; keep one ZR descriptor when padding so no
        # instruction is ever empty (count = min(n+1, N))
        cnt_k = np.bincount(g, minlength=NT * NCH).astype(np.int32)
        cnt_out = np.empty(NT * NCH, np.int32)
        for ti in range(NT):
            for c in range(NCH):
                n = int(cnt_k[ti * NCH + c])
                Ntc = int(N[ti, c])
                if n < Ntc:
                    idx_arr[128 * pbase[ti] + boff[ti, c] + n] = ZR
                    cnt_out[ti * NCH + c] = n + 1
                else:
                    cnt_out[ti * NCH + c] = n
        iw = np.tile(np.ascontiguousarray(idx_arr.reshape(-1, 16).T), (8, 1))
        rankv = np.ascontiguousarray(rank_arr.reshape(-1, 128).T.astype(BF))
        idxs.append(np.ascontiguousarray(iw))
        rankvs.append(rankv)
        cnts.append(cnt_out.reshape(1, NT * NCH))
    return dict(
        N=N, boff=boff, slots=slots, TP=TP, pbase=pbase, TOT=TOT,
        idxs=idxs, rankvs=rankvs, cnts=cnts,
    )


def _prep_host(x, edge_src, edge_dst):
    es = np.asarray(edge_src).astype(np.int64)
    ed = np.asarray(edge_dst).astype(np.int64)
    k = ed // NPC
    d = ed - k * NPC
    t = d >> 7
    r = (d & 127).astype(np.float32)

    c1 = es // CH
    l1 = es - c1 * CH
    sk = es // NPC
    hrow = sk * SL + (es - sk * NPC)
    c2 = hrow // CH
    l2 = hrow - c2 * CH

    plan1 = _plan(k, t, r, c1, l1, es)
    plan2 = _plan(k, t, r, c2, l2, hrow)

    x = np.ascontiguousarray(np.asarray(x, np.float32))
    xpad = np.zeros((NCH * CH, P), np.float32)
    xpad[:N_NODES] = x
    xc = np.zeros((NCH, CH + 1, P), np.float32)
    for c in range(NCH):
        xc[c, :CH] = xpad[c * CH : (c + 1) * CH]

    xTs = []
    for kk in range(N_CORES):
        xT = np.zeros((P, SL), np.float32)
        xT[:, :NPC] = x[kk * NPC : (kk + 1) * NPC].T
        xTs.append(np.ascontiguousarray(xT))

    return plan1, plan2, xc, xTs


SWIN = 3  # S-build lookahead window (tiles)


def _ring_params(plan, elem_bytes):
    MAXSLAB = int(plan["slots"].max())
    RING = max(4, min(12, (44 * 1024) // (MAXSLAB * elem_bytes)))
    TP = plan["TP"]
    SR = 0
    for t in range(NT):
        SR = max(SR, int(TP[max(0, t - (SWIN - 1)) : t + 1].sum()))
    SRING = SR + 2
    return MAXSLAB, RING, SRING


def _emit_gathers(gpsimd, plan, ring_s, iw_s, chunk_tabs, gsems, agd, RING,
                  MAXSLAB, cnt_s, reg):
    N, boff, pbase = plan["N"], plan["boff"], plan["pbase"]
    for t in range(NT):
        if t >= RING:
            gpsimd.wait_ge(agd, t - RING + 1)
        base = (t % RING) * MAXSLAB
        for c in range(NCH):
            n = int(N[t, c])
            ic = int((128 * pbase[t] + boff[t, c]) // 16)
            bo = int(boff[t, c])
            i = t * NCH + c
            gpsimd.load(reg, cnt_s[0:1, i : i + 1])
            gpsimd.dma_gather(
                ring_s[:, base + bo : base + bo + n].rearrange(
                    "p (j e) -> p j e", e=P
                ),
                chunk_tabs[c][:],
                iw_s[:, ic : ic + n // 16],
                n,
                reg,
                P,
                transpose=False,
                single_packet=False,
                queue_num=c,
            ).then_inc(gsems[c][t % RING], 16)


def _emit_sbuild(vector, plan, S_s, rkv_s, iota_s, t, SRING, ssem):
    TP, pbase = plan["TP"], plan["pbase"]
    for j in range(int(TP[t])):
        pi = int(pbase[t]) + j
        sl = (pi % SRING) * P
        vector.tensor_tensor(
            out=S_s[:, sl : sl + P],
            in0=rkv_s[:, pi : pi + 1].to_broadcast([P, P]),
            in1=iota_s[:],
            op=mybir.AluOpType.is_equal,
        ).then_inc(ssem, 1)


def _emit_pieces(tensor, plan, ring_s, S_s, agg_ps, t, RING, MAXSLAB, SRING,
                 ssem, agd):
    TP, pbase = plan["TP"], plan["pbase"]
    base = (t % RING) * MAXSLAB
    ntp = int(TP[t])
    for j in range(ntp):
        pi = int(pbase[t]) + j
        tensor.wait_ge(ssem, pi + 1)
        mm = tensor.matmul(
            out=agg_ps[t % 2][:],
            lhsT=ring_s[:, base + j * P : base + (j + 1) * P],
            rhs=S_s[:, (pi % SRING) * P : (pi % SRING) * P + P],
            start=(j == 0),
            stop=(j == ntp - 1),
        )
    mm.then_inc(agd, 1)


def _build_nc1(plan1):
    MAXSLAB, RING, SRING = _ring_params(plan1, 4)
    TOT, NP1 = plan1["TOT"], int(plan1["pbase"][-1])

    nc = bacc.Bacc(
        "TRN2", target_bir_lowering=False, debug=False,
        num_devices=N_CORES, num_swdge_queues=4,
    )
    xc_d = [
        nc.dram_tensor(f"xc{c}", [CH + 1, P], F32, kind="ExternalInput")
        for c in range(NCH)
    ]
    xT_d = nc.dram_tensor("xT", [P, SL], F32, kind="ExternalInput")
    iw_d = nc.dram_tensor("i1w", [P, TOT // 16], I16, kind="ExternalInput")
    rkv_d = nc.dram_tensor("rkv", [P, NP1], BF16, kind="ExternalInput")
    iota_d = nc.dram_tensor("iota", [P, P], BF16, kind="ExternalInput")
    ident_d = nc.dram_tensor("ident", [P, P], F32, kind="ExternalInput")
    w1n_d = nc.dram_tensor("W1n", [P, P], F32, kind="ExternalInput")
    w1s_d = nc.dram_tensor("W1s", [P, P], F32, kind="ExternalInput")
    w2s_d = nc.dram_tensor("W2s", [P, NCLS], F32, kind="ExternalInput")
    b1_d = nc.dram_tensor("b1", [1, P], F32, kind="ExternalInput")
    b2_d = nc.dram_tensor("b2", [1, NCLS], F32, kind="ExternalInput")
    ones_d = nc.dram_tensor("ones", [1, P], F32, kind="ExternalInput")
    cnt_d = nc.dram_tensor("cnt", [1, NT * NCH], mybir.dt.int32,
                           kind="ExternalInput")
    h_d = nc.dram_tensor("h", [SL, P], F32, kind="ExternalOutput")
    o2_d = nc.dram_tensor("o2", [P, NT * NCLS], F32, kind="ExternalOutput")

    from contextlib import ExitStack
    with ExitStack() as ctx:
        block = ctx.enter_context(nc.Block())
        sb = lambda *a: ctx.enter_context(nc.sbuf_tensor(*a))
        ps = lambda *a: ctx.enter_context(nc.psum_tensor(*a))
        sem = lambda n: ctx.enter_context(nc.semaphore(n))
        iw_s = sb("iw_s", [P, TOT // 16], I16)
        xT_s = sb("xT_s", [P, SL], F32)
        rkv_s = sb("rkv_s", [P, NP1], BF16)
        iota_s = sb("iota_s", [P, P], BF16)
        ident_s = sb("ident_s", [P, P], F32)
        w1n_s = sb("w1n_s", [P, P], F32)
        w1s_s = sb("w1s_s", [P, P], F32)
        w2s_s = sb("w2s_s", [P, NCLS], F32)
        b1_s = sb("b1_s", [1, P], F32)
        b2_s = sb("b2_s", [1, NCLS], F32)
        ones_s = sb("ones_s", [1, P], F32)
        cnt_s = sb("cnt_s", [1, NT * NCH], mybir.dt.int32)
        ring_s = sb("ring_s", [P, RING * MAXSLAB], F32)
        S_s = sb("S_s", [P, SRING * P], F32)
        aggT_s = sb("aggT_s", [P, 2 * P], F32)
        h_s = sb("h_s", [P, 2 * P], F32)
        hT_s = sb("hT_s", [P, 2 * P], F32)
        o2_sb = sb("o2_sb", [P, NT * NCLS], F32)
        agg_ps = [ps("agg_ps0", [P, P], F32), ps("agg_ps1", [P, P], F32)]
        h_ps = [ps("h_ps0", [P, P], F32), ps("h_ps1", [P, P], F32)]
        hT_ps = [ps("hT_ps0", [P, P], F32), ps("hT_ps1", [P, P], F32)]
        o2_ps = [ps("o2_ps0", [P, NCLS], F32), ps("o2_ps1", [P, NCLS], F32)]
        ld = sem("ld"); ss = sem("ss"); agd = sem("agd")
        gsems = [
            [sem(f"gs{c}_{w}") for w in range(RING)] for c in range(NCH)
        ]
        agc = sem("agc"); hd = sem("hd"); hr = sem("hr")
        hw2 = [sem("hw0"), sem("hw1")]; ow = sem("ow")
        htd = sem("htd"); htc = sem("htc"); o2d = sem("o2d"); o2c = sem("o2c")
        msz = sem("msz")
        loads = [
            (iw_s, iw_d), (xT_s, xT_d), (rkv_s, rkv_d), (iota_s, iota_d),
            (ident_s, ident_d), (w1n_s, w1n_d), (w1s_s, w1s_d),
            (w2s_s, w2s_d), (b1_s, b1_d), (b2_s, b2_d), (ones_s, ones_d),
            (cnt_s, cnt_d),
        ]
        NL = len(loads)

        @block.sync
        def _(sync: bass.BassEngine):
            for dst, src in loads:
                sync.dma_start(dst[:], src[:]).then_inc(ld, 16)
            for t in range(NT):
                sync.wait_ge(hr, t + 1)
                sync.dma_start(
                    h_d[t * P : (t + 1) * P, :],
                    h_s[:, (t % 2) * P : (t % 2 + 1) * P],
                ).then_inc(hw2[t % 2], 16)
            sync.wait_ge(o2c, NT)
            sync.dma_start(o2_d[:], o2_sb[:]).then_inc(ow, 16)
            sync.wait_ge(hw2[0], 16 * ((NT + 1) // 2))
            sync.wait_ge(hw2[1], 16 * (NT // 2))
            sync.wait_ge(ow, 16)

        @block.gpsimd
        def _(gpsimd: bass.BassGpSimd):
            gpsimd.load_library(library_config.mlp)
            gpsimd.wait_ge(ld, 16 * NL)
            gpsimd.wait_ge(msz, 1)
            with nc.gpsimd.register() as reg:
                _emit_gathers(
                    gpsimd, plan1, ring_s, iw_s, xc_d, gsems, agd, RING,
                    MAXSLAB, cnt_s, reg,
                )

        @block.vector
        def _(vector: bass.BassVectorEngine):
            vector.memset(ring_s[:], 0.0).then_inc(msz, 1)
            vector.wait_ge(ld, 16 * NL)
            for u in range(NT + 4):
                if u < NT:
                    if u >= SWIN:
                        vector.wait_ge(agd, u - (SWIN - 1))
                    _emit_sbuild(
                        vector, plan1, S_s, rkv_s, iota_s, u, SRING, ss
                    )
                if 1 <= u <= NT:
                    t = u - 1
                    vector.wait_ge(agd, t + 1)
                    vector.tensor_copy(
                        out=aggT_s[:, (t % 2) * P : (t % 2 + 1) * P],
                        in_=agg_ps[t % 2][:],
                    ).then_inc(agc, 1)
                if 3 <= u <= NT + 2:
                    t = u - 3
                    vector.wait_ge(htd, t + 1)
                    vector.tensor_copy(
                        out=hT_s[:, (t % 2) * P : (t % 2 + 1) * P],
                        in_=hT_ps[t % 2][:],
                    ).then_inc(htc, 1)
                if 4 <= u <= NT + 3:
                    t = u - 4
                    vector.wait_ge(o2d, t + 1)
                    vector.tensor_copy(
                        out=o2_sb[:, t * NCLS : (t + 1) * NCLS],
                        in_=o2_ps[t % 2][:],
                    ).then_inc(o2c, 1)

        @block.tensor
        def _(tensor: bass.BassEngine):
            tensor.wait_ge(ld, 16 * NL)
            for u in range(NT + 3):
                if u < NT:
                    t = u
                    if t >= 2:
                        tensor.wait_ge(agc, t - 1)
                    for c in range(NCH):
                        tensor.wait_ge(
                            gsems[c][t % RING], 16 * (t // RING + 1)
                        )
                    _emit_pieces(
                        tensor, plan1, ring_s, S_s, agg_ps, t, RING,
                        MAXSLAB, SRING, ss, agd,
                    )
                if 1 <= u <= NT:
                    t = u - 1
                    tensor.wait_ge(agc, t + 1)
                    if t >= 2:
                        tensor.wait_ge(hr, t - 1)
                    hps = h_ps[t % 2][:]
                    tensor.matmul(
                        out=hps,
                        lhsT=aggT_s[:, (t % 2) * P : (t % 2 + 1) * P],
                        rhs=w1n_s[:], start=True, stop=False,
                    )
                    tensor.matmul(
                        out=hps, lhsT=xT_s[:, t * P : (t + 1) * P],
                        rhs=w1s_s[:], start=False, stop=False,
                    )
                    tensor.matmul(
                        out=hps, lhsT=ones_s[:1, :], rhs=b1_s[:1, :],
                        start=False, stop=True,
                    ).then_inc(hd, 1)
                if 2 <= u <= NT + 1:
                    t = u - 2
                    tensor.wait_ge(hr, t + 1)
                    if t >= 2:
                        tensor.wait_ge(htc, t - 1)
                    # transpose h via matmul with identity rhs:
                    # out[m,n] = sum_p h[p,m] * I[p,n] = h[n,m]
                    tensor.matmul(
                        out=hT_ps[t % 2][:],
                        lhsT=h_s[:, (t % 2) * P : (t % 2 + 1) * P],
                        rhs=ident_s[:], start=True, stop=True,
                    ).then_inc(htd, 1)
                if 3 <= u <= NT + 2:
                    t = u - 3
                    tensor.wait_ge(htc, t + 1)
                    if t >= 2:
                        tensor.wait_ge(o2c, t - 1)
                    ops = o2_ps[t % 2][:]
                    tensor.matmul(
                        out=ops,
                        lhsT=hT_s[:, (t % 2) * P : (t % 2 + 1) * P],
                        rhs=w2s_s[:], start=True, stop=False,
                    )
                    tensor.matmul(
                        out=ops, lhsT=ones_s[:1, :], rhs=b2_s[:1, :],
                        start=False, stop=True,
                    ).then_inc(o2d, 1)

        @block.scalar
        def _(scalar: bass.BassEngine):
            for t in range(NT):
                scalar.wait_ge(hd, t + 1)
                if t >= 2:
                    scalar.wait_ge(hw2[t % 2], 16 * ((t - 2) // 2 + 1))
                    scalar.wait_ge(htd, t - 1)
                scalar.activation(
                    out=h_s[:, (t % 2) * P : (t % 2 + 1) * P],
                    in_=h_ps[t % 2][:],
                    func=mybir.ActivationFunctionType.Relu,
                ).then_inc(hr, 1)

    nc.compile()
    return nc


def _build_nc2(plan2):
    MAXSLAB, RING, SRING = _ring_params(plan2, 4)
    TOT, NP2 = plan2["TOT"], int(plan2["pbase"][-1])

    nc = bacc.Bacc(
        "TRN2", target_bir_lowering=False, debug=False,
        num_devices=N_CORES, num_swdge_queues=4,
    )
    hc_d = [
        nc.dram_tensor(f"hc{c}", [CH + 1, P], F32, kind="ExternalInput")
        for c in range(NCH)
    ]
    iw_d = nc.dram_tensor("i2w", [P, TOT // 16], I16, kind="ExternalInput")
    rkv_d = nc.dram_tensor("rkv2", [P, NP2], F32, kind="ExternalInput")
    iota_d = nc.dram_tensor("iota", [P, P], F32, kind="ExternalInput")
    w2n_d = nc.dram_tensor("W2n", [P, NCLS], F32, kind="ExternalInput")
    o2_d = nc.dram_tensor("o2", [P, NT * NCLS], F32, kind="ExternalInput")
    cnt_d = nc.dram_tensor("cnt2", [1, NT * NCH], mybir.dt.int32,
                           kind="ExternalInput")
    out_d = nc.dram_tensor("out", [P, NT * NCLS], F32, kind="ExternalOutput")

    from contextlib import ExitStack
    with ExitStack() as ctx:
        block = ctx.enter_context(nc.Block())
        sb = lambda *a: ctx.enter_context(nc.sbuf_tensor(*a))
        ps = lambda *a: ctx.enter_context(nc.psum_tensor(*a))
        sem = lambda n: ctx.enter_context(nc.semaphore(n))
        iw_s = sb("iw_s", [P, TOT // 16], I16)
        rkv_s = sb("rkv_s", [P, NP2], F32)
        iota_s = sb("iota_s", [P, P], F32)
        w2n_s = sb("w2n_s", [P, NCLS], F32)
        o2_sb = sb("o2_sb", [P, NT * NCLS], F32)
        cnt_s = sb("cnt_s", [1, NT * NCH], mybir.dt.int32)
        ring_s = sb("ring_s", [P, RING * MAXSLAB], F32)
        S_s = sb("S_s", [P, SRING * P], F32)
        aggH_s = sb("aggH_s", [P, 2 * P], F32)
        a_sb = sb("a_sb", [P, NT * NCLS], F32)
        ex_sb = sb("ex_sb", [P, NT * NCLS], F32)
        mx_sb = sb("mx_sb", [P, NT], F32)
        sm_sb = sb("sm_sb", [P, NT], F32)
        lg_sb = sb("lg_sb", [P, NT], F32)
        agg_ps = [ps("agg_ps0", [P, P], F32), ps("agg_ps1", [P, P], F32)]
        y_ps = [ps("y_ps0", [P, NCLS], F32), ps("y_ps1", [P, NCLS], F32)]
        ld = sem("ld"); ss = sem("ss"); agd = sem("agd")
        gsems = [
            [sem(f"gs{c}_{w}") for w in range(RING)] for c in range(NCH)
        ]
        agc = sem("agc"); yd = sem("yd"); dva = sem("dva"); dvm = sem("dvm")
        ae = sem("ae"); dvs = sem("dvs"); al = sem("al"); dvf = sem("dvf")
        mxd = sem("mxd"); msz = sem("msz")
        ow = sem("ow")
        loads = [
            (iw_s, iw_d), (rkv_s, rkv_d), (iota_s, iota_d),
            (w2n_s, w2n_d), (o2_sb, o2_d), (cnt_s, cnt_d),
        ]
        NL = len(loads)

        @block.sync
        def _(sync: bass.BassEngine):
            for dst, src in loads:
                sync.dma_start(dst[:], src[:]).then_inc(ld, 16)
            sync.wait_ge(dvf, 1)
            sync.dma_start(out_d[:], a_sb[:]).then_inc(ow, 16)
            sync.wait_ge(ow, 16)

        @block.gpsimd
        def _(gpsimd: bass.BassGpSimd):
            gpsimd.load_library(library_config.mlp)
            gpsimd.wait_ge(ld, 16 * NL)
            gpsimd.wait_ge(msz, 1)
            with nc.gpsimd.register() as reg:
                _emit_gathers(
                    gpsimd, plan2, ring_s, iw_s, hc_d, gsems, agd, RING,
                    MAXSLAB, cnt_s, reg,
                )

        @block.vector
        def _(vector: bass.BassVectorEngine):
            vector.memset(ring_s[:], 0.0).then_inc(msz, 1)
            vector.wait_ge(ld, 16 * NL)
            for u in range(NT + 2):
                if u < NT:
                    if u >= SWIN:
                        vector.wait_ge(agd, u - (SWIN - 1))
                    _emit_sbuild(
                        vector, plan2, S_s, rkv_s, iota_s, u, SRING, ss
                    )
                if 1 <= u <= NT:
                    t = u - 1
                    vector.wait_ge(agd, t + 1)
                    vector.tensor_copy(
                        out=aggH_s[:, (t % 2) * P : (t % 2 + 1) * P],
                        in_=agg_ps[t % 2][:],
                    ).then_inc(agc, 1)
                if 2 <= u <= NT + 1:
                    t = u - 2
                    vector.wait_ge(yd, t + 1)
                    vector.tensor_tensor(
                        out=a_sb[:, t * NCLS : (t + 1) * NCLS],
                        in0=y_ps[t % 2][:],
                        in1=o2_sb[:, t * NCLS : (t + 1) * NCLS],
                        op=mybir.AluOpType.add,
                    ).then_inc(dva, 1)
            # log-softmax tail over the whole [P, NT, NCLS] array
            vector.wait_ge(dva, NT)
            a3 = a_sb[:].rearrange("p (t c) -> p t c", c=NCLS)
            vector.tensor_reduce(
                out=mx_sb[:], in_=a3, axis=mybir.AxisListType.X,
                op=mybir.AluOpType.max,
            ).then_inc(mxd, 1)
            vector.wait_ge(mxd, 1)
            mxb = mx_sb[:].unsqueeze(2).to_broadcast([P, NT, NCLS])
            vector.tensor_tensor(
                out=a3, in0=a3, in1=mxb, op=mybir.AluOpType.subtract
            ).then_inc(dvm, 1)
            vector.wait_ge(ae, 1)
            vector.tensor_reduce(
                out=sm_sb[:],
                in_=ex_sb[:].rearrange("p (t c) -> p t c", c=NCLS),
                axis=mybir.AxisListType.X,
                op=mybir.AluOpType.add,
            ).then_inc(dvs, 1)
            vector.wait_ge(al, 1)
            lgb = lg_sb[:].unsqueeze(2).to_broadcast([P, NT, NCLS])
            vector.tensor_tensor(
                out=a3, in0=a3, in1=lgb, op=mybir.AluOpType.subtract
            ).then_inc(dvf, 1)

        @block.tensor
        def _(tensor: bass.BassEngine):
            tensor.wait_ge(ld, 16 * NL)
            for u in range(NT + 1):
                if u < NT:
                    t = u
                    if t >= 2:
                        tensor.wait_ge(agc, t - 1)
                    for c in range(NCH):
                        tensor.wait_ge(
                            gsems[c][t % RING], 16 * (t // RING + 1)
                        )
                    _emit_pieces(
                        tensor, plan2, ring_s, S_s, agg_ps, t, RING,
                        MAXSLAB, SRING, ss, agd,
                    )
                if 1 <= u <= NT:
                    t = u - 1
                    tensor.wait_ge(agc, t + 1)
                    if t >= 2:
                        tensor.wait_ge(dva, t - 1)
                    tensor.matmul(
                        out=y_ps[t % 2][:],
                        lhsT=aggH_s[:, (t % 2) * P : (t % 2 + 1) * P],
                        rhs=w2n_s[:], start=True, stop=True,
                    ).then_inc(yd, 1)

        @block.scalar
        def _(scalar: bass.BassEngine):
            scalar.wait_ge(dvm, 1)
            scalar.activation(
                out=ex_sb[:], in_=a_sb[:],
                func=mybir.ActivationFunctionType.Exp,
            ).then_inc(ae, 1)
            scalar.wait_ge(dvs, 1)
            scalar.activation(
                out=lg_sb[:], in_=sm_sb[:],
                func=mybir.ActivationFunctionType.Ln,
            ).then_inc(al, 1)

    nc.compile()
    return nc


def _common_inputs1(plan1, xc, xTs, W_neigh1, W_self1, b1, W_neigh2, W_self2,
                    b2):
    iota = np.ascontiguousarray(
        np.broadcast_to(np.arange(P, dtype=np.float32), (P, P)).astype(BF)
    )
    ident = np.eye(P, dtype=np.float32)
    common = {
        **{f"xc{c}": np.ascontiguousarray(xc[c]) for c in range(NCH)},
        "iota": iota,
        "ident": ident,
        "W1n": np.asarray(W_neigh1, np.float32),
        "W1s": np.asarray(W_self1, np.float32),
        "W2s": np.asarray(W_self2, np.float32),
        "b1": np.asarray(b1, np.float32).reshape(1, P),
        "b2": np.asarray(b2, np.float32).reshape(1, NCLS),
        "ones": np.ones((1, P), np.float32),
    }
    return [
        {
            **common,
            "xT": xTs[k],
            "i1w": plan1["idxs"][k],
            "rkv": plan1["rankvs"][k],
            "cnt": plan1["cnts"][k],
        }
        for k in range(N_CORES)
    ]


def _inputs2(plan2, hc, o2s, W_neigh2):
    iota = np.ascontiguousarray(
        np.broadcast_to(np.arange(P, dtype=np.float32), (P, P)).copy()
    )
    common = {
        **{f"hc{c}": np.ascontiguousarray(hc[c]) for c in range(NCH)},
        "iota": iota,
        "W2n": np.asarray(W_neigh2, np.float32),
    }
    return [
        {
            **common,
            "i2w": plan2["idxs"][k],
            "rkv2": plan2["rankvs"][k].astype(np.float32),
            "cnt2": plan2["cnts"][k],
            "o2": o2s[k],
        }
        for k in range(N_CORES)
    ]


def _h_chunks(h_list):
    htab = np.concatenate(h_list, axis=0)  # [8*SL, P] == [4*CH, P]
    hc = np.zeros((NCH, CH + 1, P), np.float32)
    for c in range(NCH):
        hc[c, :CH] = htab[c * CH : (c + 1) * CH]
    return hc


def kernel(
    x, edge_src, edge_dst, W_neigh1, W_self1, b1, W_neigh2, W_self2, b2
):
    plan1, plan2, xc, xTs = _prep_host(x, edge_src, edge_dst)

    in_maps1 = _common_inputs1(
        plan1, xc, xTs, W_neigh1, W_self1, b1, W_neigh2, W_self2, b2
    )
    nc1 = _build_nc1(plan1)
    res1 = run_bass_kernel_spmd(nc1, in_maps1, list(range(N_CORES)))

    hc = _h_chunks([res1.results[k]["h"] for k in range(N_CORES)])
    in_maps2 = _inputs2(
        plan2, hc, [res1.results[k]["o2"] for k in range(N_CORES)], W_neigh2
    )
    nc2 = _build_nc2(plan2)
    res2 = run_bass_kernel_spmd(nc2, in_maps2, list(range(N_CORES)))

    out_full = np.empty((N_NODES, NCLS), dtype=np.float32)
    for k in range(N_CORES):
        o = res2.results[k]["out"]
        rows = o.reshape(P, NT, NCLS).transpose(1, 0, 2).reshape(SL, NCLS)
        out_full[k * NPC : (k + 1) * NPC] = rows[:NPC]
    return out_full


if __name__ == "__main__":
    import jax

    import reference

    cpu = jax.devices("cpu")[0]
    with jax.default_device(cpu):
        inputs = {
            k: np.asarray(v) for k, v in reference.setup_inputs().items()
        }
        exp = np.asarray(
            reference.reference(
                **{k: jax.device_put(v, cpu) for k, v in inputs.items()}
            )
        )
    got = kernel(**inputs)
    err = np.abs(got - exp)
    denom = np.maximum(np.abs(exp), 1e-3)
    print("max abs err:", err.max(), "max rel err:", (err / denom).max())
